# revision 1
# baseline (speedup 1.0000x reference)
"""Trainium2 Bass kernel for nn_CompactControlAttention.

The module's attention is degenerate: softmax over a size-1 axis is exactly
1.0, so queries/keys (Wq, bq, Wk, bk) never affect the output:

    out[b, s, :] = sequence[b, s, :] + p[b, :]
    p = (sum_c controls[c]) @ Wv.T @ Wo.T + C * (bv @ Wo.T + bo)

Sharding (8 cores, no collectives): tensor-parallel over the OUTPUT feature
dim e. Core k computes out[:, :, 256k:256(k+1)], which needs full Wv
(replicated), a 256-column slice of Wo, and the matching slice of
sequence/bo. Weight matrices are shipped pre-transposed ([in, out] layout)
so the contraction dim lands on SBUF partitions without on-device
transposes of the big weights.

Per-core device program:
  cs = sum_c controls[c]                 (DVE tree + last fold fused below)
  csT = cs.T                             (16 matmuls against stacked identity)
  v = cs @ Wv.T + C*bv                   (PSUM accum over 16 K-tiles)
  vT = v.T                               (16 PE transposes)
  p = v @ WoT_k + bo                     (col-tiled into a [128,128] PSUM tile)
  out = seq_k + broadcast_s(p)           (free-dim step-0 broadcast on DVE)

MM_DT selects TensorEngine precision for the two big GEMMs:
  bf16: weights rounded to bf16 host-side, activations cast on-device,
    fp32 PSUM accumulation. ~1e-3 rel err, fastest (half DMA + 1 cyc/row).
  f32r: tf32-like 2xbf16 decomposition, ~1e-4 rel err, ~2 cyc/row.
  f32: exact fp32, ~4e-7 rel err, 4 cyc/row.
"""

import numpy as np
import ml_dtypes

import concourse.bass as bass
import concourse.mybir as mybir
import concourse.tile as tile
from concourse import bacc
from concourse.bass_utils import run_bass_kernel_spmd
from concourse.masks import make_identity

N_CORES = 8
D = 2048
B = 64
S = 32
C = 8
EK = D // N_CORES  # 256: output-feature slice per core
F32 = mybir.dt.float32
F32R = mybir.dt.float32r
BF16 = mybir.dt.bfloat16

MM_DT = "bf16"  # "bf16" | "f32r" | "f32"

_CACHE = {}


def _build_nc(mm_dt):
    w_dt = BF16 if mm_dt == "bf16" else F32  # DRAM dtype of shipped weights
    nc = bacc.Bacc("TRN2", target_bir_lowering=False, debug=False, num_devices=N_CORES)

    seq = nc.dram_tensor("seq", [128, S * 128], F32, kind="ExternalInput")
    ctrl = nc.dram_tensor("ctrl", [C * B, D], F32, kind="ExternalInput")
    wvt = nc.dram_tensor("wvt", [D, D], w_dt, kind="ExternalInput")  # Wv.T [f, d]
    wot = nc.dram_tensor("wot", [D, EK], w_dt, kind="ExternalInput")  # Wo.T[:, e_k]
    bv = nc.dram_tensor("bv", [D], F32, kind="ExternalInput")
    bo = nc.dram_tensor("bo", [EK], F32, kind="ExternalInput")
    out = nc.dram_tensor("out", [128, S * 128], F32, kind="ExternalOutput")

    with tile.TileContext(nc) as tc:
        _body(tc, seq, ctrl, wvt, wot, bv, bo, out, mm_dt)
    nc.compile()
    return nc


def _body(tc, seq, ctrl, wvt, wot, bv, bo, out, mm_dt):
    from contextlib import ExitStack

    mdt = {"bf16": BF16, "f32r": F32R, "f32": F32}[mm_dt]

    ctx = ExitStack()
    nc = tc.nc
    P = 128

    consts = ctx.enter_context(tc.tile_pool(name="consts", bufs=1))
    sbuf = ctx.enter_context(tc.tile_pool(name="sbuf", bufs=1))
    wpool = ctx.enter_context(tc.tile_pool(name="wv", bufs=3))
    psum_t = ctx.enter_context(tc.tile_pool(name="psum_t", bufs=2, space="PSUM"))
    psum_v = ctx.enter_context(tc.tile_pool(name="psum_v", bufs=1, space="PSUM"))
    psum_p = ctx.enter_context(tc.tile_pool(name="psum_p", bufs=1, space="PSUM"))

    # --- controls first: split across both DMA queues (SWDGE + HWDGE) so
    # the cs chain -- the kernel's critical-path prefix -- is not starved
    # behind weight traffic (both queue FIFOs process it first).
    ctrl_sb = sbuf.tile([P, 4 * D], F32)
    nc.gpsimd.dma_start(
        out=ctrl_sb[:, 0 : 2 * D].rearrange("p (g d) -> p g d", d=D),
        in_=ctrl[0 : 2 * P, :].rearrange("(g p) d -> p g d", p=P),
    )
    nc.sync.dma_start(
        out=ctrl_sb[:, 2 * D : 4 * D].rearrange("p (g d) -> p g d", d=D),
        in_=ctrl[2 * P : 4 * P, :].rearrange("(g p) d -> p g d", p=P),
    )

    # --- constants -------------------------------------------------------
    ident = consts.tile([P, P], F32)
    make_identity(nc, ident[:])
    # sel = two stacked 64x64 identities: a matmul against sel folds the
    # last c-parity pair while transposing.
    sel = consts.tile([P, B], F32)
    nc.gpsimd.dma_start(out=sel[0:B, :], in_=ident[0:B, 0:B])
    nc.gpsimd.dma_start(out=sel[B : 2 * B, :], in_=ident[0:B, 0:B])
    ones8_f = consts.tile([1, B], F32)
    nc.vector.memset(ones8_f[:], float(C))
    ones1_f = consts.tile([1, B], F32)
    nc.vector.memset(ones1_f[:], 1.0)
    ones8 = consts.tile([1, B], mdt)  # value C: bias-augment row for MM1
    nc.vector.tensor_copy(ones8[:], ones8_f[:])
    ones1 = consts.tile([1, B], mdt)  # value 1: bias-augment row for MM2
    nc.vector.tensor_copy(ones1[:], ones1_f[:])

    ident_t = ident if mdt == F32 else consts.tile([P, P], mdt, name="ident_t")
    if mdt != F32:
        nc.vector.tensor_copy(ident_t[:], ident[:])

    # --- fold controls over C --------------------------------------------
    acc = sbuf.tile([P, D], F32)
    nc.vector.tensor_add(acc[:], ctrl_sb[:, 0:D], ctrl_sb[:, D : 2 * D])
    nc.vector.tensor_add(acc[:], acc[:], ctrl_sb[:, 2 * D : 3 * D])
    nc.vector.tensor_add(acc[:], acc[:], ctrl_sb[:, 3 * D : 4 * D])

    # --- csT: fold last c-pair + transpose in one matmul per f-block -----
    cst = sbuf.tile([P, 16 * B], mdt)  # block j at cols [64j, 64j+64)
    for j in range(16):
        pt = psum_t.tile([P, B], F32, tag="pt")
        nc.tensor.matmul(
            pt[:], acc[:, j * P : (j + 1) * P], sel[:], start=True, stop=True
        )
        nc.vector.tensor_copy(cst[:, j * B : (j + 1) * B], pt[:])

    # --- MM1: v = cs @ Wv.T + C*bv  (v in 4 PSUM banks of [64, 512]) -----
    # f32r needs a rounding producer: SWDGE cast-DMA. bf16/f32 ship native.
    wv_dma = nc.gpsimd.dma_start if mm_dt == "f32r" else nc.sync.dma_start
    pv = [psum_v.tile([B, 512], F32, tag=f"pv{c}", name=f"pv{c}") for c in range(4)]
    for jj in range(8):  # stream Wv.T in chunks of two 128-row tiles
        wv_sb = wpool.tile([P, 2 * D], mdt)
        wv_dma(
            out=wv_sb[:].rearrange("p (g d) -> p g d", d=D),
            in_=wvt[jj * 256 : (jj + 1) * 256, :].rearrange("(g p) d -> p g d", p=P),
        )
        for g in range(2):
            j = 2 * jj + g
            for c in range(4):
                nc.tensor.matmul(
                    pv[c][:],
                    cst[:, j * B : (j + 1) * B],
                    wv_sb[:, g * D + c * 512 : g * D + (c + 1) * 512],
                    start=(j == 0),
                    stop=False,
                )
    bv_sb = consts.tile([1, D], mdt)
    nc.gpsimd.dma_start(out=bv_sb[:], in_=bv[None, :])
    for c in range(4):  # bias-augment row: += C * bv
        nc.tensor.matmul(
            pv[c][:],
            ones8[:],
            bv_sb[:, c * 512 : (c + 1) * 512],
            start=False,
            stop=True,
        )
    v = sbuf.tile([B, D], mdt)
    for c in range(4):
        nc.vector.tensor_copy(v[:, c * 512 : (c + 1) * 512], pv[c][:])

    # --- late inputs: issued after the wv stream in each queue FIFO ------
    wo_sb = sbuf.tile([P, 16 * EK], mdt)  # d-tile t at cols [256t, 256t+256)
    wo_dma = nc.gpsimd.dma_start if mm_dt == "f32r" else nc.sync.dma_start
    wo_dma(
        out=wo_sb[:].rearrange("p (t e) -> p t e", e=EK),
        in_=wot.rearrange("(t p) e -> p t e", p=P),
    )
    bo_sb = consts.tile([1, EK], mdt)
    nc.gpsimd.dma_start(out=bo_sb[:], in_=bo[None, :])
    seq_sb = sbuf.tile([P, S * 128], F32)
    nc.sync.dma_start(out=seq_sb[:], in_=seq[:])

    # --- vT: 16 PE transposes --------------------------------------------
    vt = sbuf.tile([P, 16 * B], mdt)
    for t in range(16):
        pt = psum_t.tile([P, B], mdt, name="ptv", tag="pt")
        nc.tensor.transpose(pt[:], v[:, t * P : (t + 1) * P], ident_t[0:B, 0:B])
        nc.vector.tensor_copy(vt[:, t * B : (t + 1) * B], pt[:])

    # --- MM2: p = v @ WoT_k + bo, col-tiled into [128, 128] --------------
    pp = psum_p.tile([P, P], F32, tag="pp")
    for half in range(2):
        o = pp[half * B : (half + 1) * B, :]
        for t in range(16):
            nc.tensor.matmul(
                o,
                vt[:, t * B : (t + 1) * B],
                wo_sb[:, t * EK + half * P : t * EK + (half + 1) * P],
                start=(t == 0),
                stop=False,
            )
        nc.tensor.matmul(
            o, ones1[:], bo_sb[:, half * P : (half + 1) * P], start=False, stop=True
        )
    p_re = sbuf.tile([P, P], F32)
    nc.vector.tensor_copy(p_re[:], pp[:])

    # --- sequence + broadcast(p) -----------------------------------------
    # seq layout (host-prepared): partition p = 64*eh + b, free = (s, e');
    # p broadcasts along the free s-dim (step-0), which DVE supports.
    out_sb = sbuf.tile([P, S * 128], F32)
    nc.vector.tensor_add(
        out_sb[:].rearrange("p (s e) -> p s e", e=P),
        seq_sb[:].rearrange("p (s e) -> p s e", e=P),
        p_re[:, None, :].to_broadcast((P, S, P)),
    )
    nc.sync.dma_start(out=out[:], in_=out_sb[:])
    ctx.close()


def _get_nc(mm_dt=None):
    mm_dt = mm_dt or MM_DT
    if mm_dt not in _CACHE:
        _CACHE[mm_dt] = _build_nc(mm_dt)
    return _CACHE[mm_dt]


def _shard(sequence, controls, Wv, bv, Wo, bo, mm_dt):
    wnp = ml_dtypes.bfloat16 if mm_dt == "bf16" else np.float32
    wvt = np.ascontiguousarray(Wv.T.astype(wnp))
    ctrl = np.ascontiguousarray(controls.reshape(C * B, D))
    in_maps = []
    for k in range(N_CORES):
        ek = slice(k * EK, (k + 1) * EK)
        in_maps.append(
            {
                "seq": np.ascontiguousarray(
                    sequence[:, :, ek]
                    .reshape(B, S, 2, 128)
                    .transpose(2, 0, 1, 3)
                    .reshape(128, S * 128)
                ),
                "ctrl": ctrl,
                "wvt": wvt,
                "wot": np.ascontiguousarray(Wo[ek, :].T.astype(wnp)),
                "bv": np.ascontiguousarray(bv),
                "bo": np.ascontiguousarray(bo[ek]),
            }
        )
    return in_maps


def _run(inputs, trace=False, mm_dt=None):
    mm_dt = mm_dt or MM_DT
    nc = _get_nc(mm_dt)
    in_maps = _shard(
        np.asarray(inputs["sequence"]), np.asarray(inputs["controls"]),
        np.asarray(inputs["Wv"]), np.asarray(inputs["bv"]),
        np.asarray(inputs["Wo"]), np.asarray(inputs["bo"]), mm_dt,
    )
    res = run_bass_kernel_spmd(nc, in_maps, list(range(N_CORES)), trace=trace)
    out = np.empty((B, S, D), dtype=np.float32)
    for k in range(N_CORES):
        out[:, :, k * EK : (k + 1) * EK] = (
            res.results[k]["out"]
            .reshape(2, B, S, 128)
            .transpose(1, 2, 0, 3)
            .reshape(B, S, EK)
        )
    return out, res


def kernel(**inputs):
    out, _ = _run(inputs)
    return out

